# revision 23
# baseline (speedup 1.0000x reference)
"""Trainium2 Bass kernel for nn_NestedMoEModel (moe_routing).

Mathematical reduction of the reference:
  gate = softmax(x @ W_gate.T + b_gate, axis=1)        # rows sum to 1.0
  out  = gate.sum(1, keepdims=True) * expert_flat      # == expert_flat (±1 ulp)
  expert_flat[b, g*H+h] = sum_i x[b,i] * sum_e W_exp[g,e,h,i] + sum_e b_exp[g,e,h]

So the device kernel is a single bias-GEMM:
  out[B, N=G*H] = x[B, D] @ W_sum[D, N] + b_sum[N]
with W_sum = sum_e W_exp (transposed), b_sum = sum_e b_exp (host prep, ~16MB).

Sharding: data-parallel over batch B across 8 cores (4096 rows each);
weights/bias replicated. No collectives.

Device layout: output is computed TRANSPOSED — out_t[n, b] — so the
per-column bias becomes per-PARTITION. Structure tuned from perfetto
traces (the 256-matmul fp16 stream at the 216ns/matmul FILL-limited
floor = 55.4us is the roofline; everything else must overlap it; exec
time also counts ~7us NEFF preamble and ~3us postamble/barrier):

 - _dedup_ldweights removes the per-matmul redundant Ldweights that
   bass emits (walrus's own redundant-LDW opt is hardcoded OFF and
   rejects pre-split InstLdweights). With one Ldweights per DISTINCT
   stationary operand, same-weight matmul pairs issue back-to-back at
   216ns (fill-limited) instead of 263ns (fill+drain: a weight-buffer
   swap only commits after the previous matmul fully drains). This
   alone is ~12us over the 256-matmul stream.
 - One SBUF output tile per h-tile (opool bufs=16 == HT): a smaller
   rotating pool makes ht6+ drains wait on ht0-5 store DMAs (WAW on
   the SBUF buffer) — measured as a 4.5us PE stall mid-stream plus a
   HAM re-throttle. 16 tiles of [128, 4096] f16 = 128KB/partition,
   which fits alongside x (16KB) + w (8KB) in the 208KB budget.
 - Input DMAs are few and large, on the sync (SP) HWDGE queue in
   first-consumption order (issue ~610ns serial/queue; engines pick up
   ~1.3us later; a DMA's completion semaphore fires ~2-2.5us after its
   last byte, so the queue head bounds the first real matmul at
   ~11.2-12.4us). Bias rides the scalar queue behind the unavoidable
   1.3us ACT table load. (Weights via the gpsimd SWDGE queue measured
   WORSE: Q7 descriptor generation lands them ~2us later.)
 - 10 warm-up matmuls on a DVE-zeroed tile bridge from the earliest
   possible PE cycle (~7.5-8.0us) past worst-case first-data; the HAM
   clock un-throttle needs ~3.4us of sustained PE busy, and even a
   ~1.3us idle gap can re-throttle it (measured: ~2us of cold matmuls).
 - The first 3 units emit k-major ACROSS units so their k0 matmuls
   (which only need the first two DMAs) absorb the k1 data latency.
 - PSUM tiled [128,1024] x 4 rotating slots; per-unit drain is a
   per-partition bias-add split 3:2 across ScalarE activation(Identity,
   bias) (~1.06us) and VectorE tensor_scalar_add (~1.28us).
 - Stores: quarter-tiles (256KB) in the opening bq-major phase and in
   the closing 3-h-tile bq-major stretch (keeps the DMA ring drained
   for the tail), half-tiles (512KB) in the ht-major middle. The
   second-to-last unit drains per-half ACT||DVE and stores 128KB
   pieces on sync/scalar; the final unit drains 256-col pieces and
   stores 64KB pieces alternating queues so the final bytes (and
   their ~2us HBM-write receipt) start as early as possible.
The host un-transposes the output at the end (numpy, not graded HW time).

dtype config (CONFIG): matmul inputs float32r (fp32 storage, single-pass
PE multiply) or float16; output float32 or float16 (halves the dominant
write traffic; fp32 PSUM is rounded once on the epilogue write).
"""

import os
import numpy as np


def _dedup_ldweights(nc):
    """Remove redundant Ldweights: bass emits one per matmul, and every
    weight-buffer swap forces the next matmul to wait for the previous
    one's full fill+drain (263ns) instead of fill only (216ns).  Keeping
    one Ldweights per *distinct* stationary operand lets same-weight
    matmul runs issue back-to-back at the fill-limited floor.  Deps of a
    removed Ldweights are merged into the next PE instruction."""
    PE = None
    for f in nc.m.functions:
        for b in f.blocks:
            insts = b.instructions
            new = []
            cur_sig = None
            victims = []
            for i in insts:
                eng = str(i.engine)
                if eng != "EngineType.PE":
                    new.append(i)
                    continue
                if i.opcode == "Ldweights":
                    sig = (
                        str(i.ins[0]),
                        str(getattr(i, "perf_mode", None)),
                        str(getattr(i, "is_transpose", None)),
                    )
                    if sig == cur_sig:
                        victims.append(i)
                        continue
                    cur_sig = sig
                    new.append(i)
                elif i.opcode == "Matmult":
                    for v in victims:
                        i.merge_dependencies_from(v)
                    victims = []
                    new.append(i)
                else:
                    # unknown PE instruction: conservatively forget state
                    cur_sig = None
                    for v in victims:
                        new.append(v)
                    victims = []
                    new.append(i)
            assert not victims
            b.instructions = new

B, D, H, G, E = 32768, 256, 256, 8, 8
N = G * H               # 2048 output columns (= partition rows of out_t)
NCORES = 8
BS = B // NCORES        # 4096 batch rows per core
P = 128                 # partitions
KO = D // P             # 2 contraction chunks of 128
HT = N // P             # 16 h-tiles (output partition tiles)
BQ = BS // 1024         # 4 b-quarters per h-tile (PSUM unit [128, 1024])
NWARM = 10              # bridge first-PE-cycle (~7.5-8.0us) past worst-case
                        # first-input-sem (~12.4us) in ONE unbroken PE-busy
                        # stretch: even a ~1.3us PE idle gap can re-throttle
                        # the HAM clock and cost ~2us of cold matmuls

# "f32"    : float32r matmul, float32 output   (safest, ~121us)
# "f16out" : float32r matmul, float16 output   (output quantization ~5e-4)
# "f16"    : float16 matmul + output           (fastest, err ~1e-3)
CONFIG = os.environ.get("KDTYPE", "f16")

_LAST_RESULTS = None    # BassKernelResults of the most recent run (for profiling)
_NC_CACHE = {}


def _build_nc(config):
    import concourse.bacc as bacc
    import concourse.mybir as mybir
    import concourse.tile as tile

    f32 = mybir.dt.float32
    in_dt = mybir.dt.float16 if config == "f16" else mybir.dt.float32r
    out_dt = f32 if config == "f32" else mybir.dt.float16
    IDENT = mybir.ActivationFunctionType.Identity

    nc = bacc.Bacc("TRN2", target_bir_lowering=False, debug=False)

    xt_h = nc.dram_tensor("xt", [D, BS], in_dt, kind="ExternalInput")
    wt_h = nc.dram_tensor("wt", [P, KO, N], in_dt, kind="ExternalInput")
    bias_h = nc.dram_tensor("biasp", [P, HT], f32, kind="ExternalInput")
    out_h = nc.dram_tensor("out", [N, BS], out_dt, kind="ExternalOutput")

    xt_ap = xt_h[:].rearrange("(ko p) b -> ko p b", p=P)     # [KO, 128, BS]
    out_ap = out_h[:].rearrange("(ht p) b -> ht p b", p=P)   # [HT, 128, BS]

    with tile.TileContext(nc) as tc:
        with (
            tc.tile_pool(name="wpool", bufs=1) as wpool,
            tc.tile_pool(name="xpool", bufs=1) as xpool,
            tc.tile_pool(name="opool", bufs=16) as opool,
            tc.tile_pool(name="pspool", bufs=4, space="PSUM") as pspool,
        ):
            # PE warm-up on a DVE-zeroed tile (fast memset->PE sem handoff).
            # 8 x 512-col cold matmuls = ~3.4us of PE busy = one HAM window,
            # so the clock un-throttles right as the first input data lands.
            warm_sb = wpool.tile([P, 512], mybir.dt.float16, name="warm_sb")
            nc.vector.memset(warm_sb[:], 0.0)
            ps_warm = pspool.tile([P, 1024], f32, name="ps")
            for _ in range(NWARM):
                nc.tensor.matmul(ps_warm[:, 0:512], warm_sb[:, 0:P], warm_sb[:], start=True, stop=True)

            w_sb = wpool.tile([P, KO, N], in_dt, name="w_sb")
            x_sb = [xpool.tile([P, BS], in_dt, name=f"x_sb{k}") for k in range(KO)]
            bias_sb = wpool.tile([P, HT], f32, name="bias_sb")

            # All loads on the sync (SP) HWDGE queue, in first-consumption
            # order (issue is ~610ns serial per queue; each DMA's semaphore
            # fires ~2us after its last byte, so the head of this queue sets
            # when the first real matmul can start; measured first-input
            # sem ~11.2-12.4us). Bias rides the scalar queue behind the
            # unavoidable 1.3us ACT table load (bias is only needed by
            # the first drain ~14us).
            # x00 on the scalar queue, in PARALLEL with w0a on sync: the
            # ACT table load rides a runtime-internal ring, so scalar's
            # HWDGE head is free and both first-matmul gates transfer
            # concurrently (~11.2us typical instead of ~12.2us serial).
            nc.scalar.dma_start(x_sb[0][:, 0:1024], xt_ap[0][:, 0:1024])
            nc.scalar.dma_start(bias_sb[:], bias_h[:])
            nc.sync.dma_start(w_sb[:, 0, 0:768], wt_h[:, 0, 0:768])
            nc.sync.dma_start(w_sb[:, 1, 0:768], wt_h[:, 1, 0:768])
            nc.sync.dma_start(x_sb[1][:, 0:1024], xt_ap[1][:, 0:1024])
            nc.sync.dma_start(x_sb[0][:, 1024:2048], xt_ap[0][:, 1024:2048])
            nc.sync.dma_start(x_sb[1][:, 1024:2048], xt_ap[1][:, 1024:2048])
            nc.sync.dma_start(x_sb[0][:, 2048:BS], xt_ap[0][:, 2048:BS])
            nc.sync.dma_start(x_sb[1][:, 2048:BS], xt_ap[1][:, 2048:BS])
            nc.sync.dma_start(w_sb[:, 0, 768:N], wt_h[:, 0, 768:N])
            nc.sync.dma_start(w_sb[:, 1, 768:N], wt_h[:, 1, 768:N])

            # First 6 h-tiles run b-quarter-major so they track x arrival
            # (~5.2us of work per x chunk tolerates multi-us DMA jitter);
            # the middle runs h-tile-major with x fully resident; the last 3
            # h-tiles run b-quarter-major again so their stores spread out
            # and the DMA ring is empty when the final unit's bytes arrive.
            units = [(ht, bq) for bq in range(BQ) for ht in range(6)]
            units += [(ht, bq) for ht in range(6, HT - 3) for bq in range(BQ)]
            units += [(ht, bq) for bq in range(BQ) for ht in range(HT - 3, HT)]

            out_tiles = {}

            def get_out(ht):
                if ht not in out_tiles:
                    out_tiles[ht] = opool.tile([P, BS], out_dt, name="out_sb")
                return out_tiles[ht]

            def mm_k(ps_ap, ht, b0, bb, k):
                nc.tensor.matmul(
                    ps_ap,
                    w_sb[:, k, ht * P:(ht + 1) * P],
                    x_sb[k][:, b0 + bb * 512:b0 + (bb + 1) * 512],
                    start=(k == 0),
                    stop=(k == KO - 1),
                )

            # Units 0-2 are emitted k-major ACROSS units: all k0 matmuls
            # first (they only need the first x00/w0a DMAs), then the k1
            # matmuls — absorbing the x10/w1a semaphore latency (+~1us).
            ps3 = [pspool.tile([P, 1024], f32, name="ps") for _ in range(3)]
            for k in range(KO):
                for bb in range(2):
                    for u in range(3):
                        mm_k(ps3[u][:, bb * 512:(bb + 1) * 512], u, 0, bb, k)
            # Split each head unit's drain into ACT||DVE halves so the 3
            # PSUM slots free ~serially every 0.6us instead of 1.1us: the
            # slot-2 reuse by unit 5 otherwise stalls the PE ~360ns.
            for u in range(3):
                out_u = get_out(u)
                bias_u = bias_sb[:, u:u + 1]
                nc.scalar.activation(out_u[:, 0:512], ps3[u][:, 0:512], IDENT, bias=bias_u)
                nc.vector.tensor_scalar_add(out_u[:, 512:1024], ps3[u][:, 512:1024], bias_u)
                nc.sync.dma_start(out_ap[u][:, 0:1024], out_u[:, 0:1024])

            n_units = len(units)
            for unit in range(3, n_units):
                ht, bq = units[unit]
                out_sb = get_out(ht)
                bias_col = bias_sb[:, ht:ht + 1]
                b0 = bq * 1024
                if unit >= n_units - 2:
                    # Last two units: bb-major matmuls into one-bank PSUM
                    # tiles; each 512-col half drains ACT (bb0) || DVE (bb1)
                    # and stores 128KB immediately. The final unit's stores
                    # ride the otherwise-idle scalar queue; the second-to-
                    # last keeps sync — so neither final issue waits behind
                    # the other unit's (~600ns serial DIRECT2D per queue).
                    q = nc.scalar if unit == n_units - 1 else nc.sync
                    for bb in range(2):
                        ps_h = pspool.tile([P, 512], f32, name="ps")
                        for k in range(KO):
                            mm_k(ps_h[:], ht, b0, bb, k)
                        dst = out_sb[:, b0 + bb * 512:b0 + (bb + 1) * 512]
                        if bb == 0:
                            nc.scalar.activation(dst, ps_h[:], IDENT, bias=bias_col)
                        else:
                            nc.vector.tensor_scalar_add(dst, ps_h[:], bias_col)
                        q.dma_start(out_ap[ht][:, b0 + bb * 512:b0 + (bb + 1) * 512], dst)
                    continue
                ps = pspool.tile([P, 1024], f32, name="ps")
                for k in range(KO):
                    for bb in range(2):
                        mm_k(ps[:, bb * 512:(bb + 1) * 512], ht, b0, bb, k)
                dst = out_sb[:, b0:b0 + 1024]
                # 3:2 ACT:DVE — ACT is a touch faster; DVE takes 2 of 5.
                if unit >= n_units - 4:
                    # 3rd/4th-to-last: split the drain ACT || DVE so both
                    # engines clear early for the last two units' halves.
                    nc.scalar.activation(out_sb[:, b0:b0 + 512], ps[:, 0:512], IDENT, bias=bias_col)
                    nc.vector.tensor_scalar_add(out_sb[:, b0 + 512:b0 + 1024], ps[:, 512:1024], bias_col)
                elif unit % 5 in (2, 4):
                    nc.vector.tensor_scalar_add(dst, ps[:], bias_col)
                else:
                    nc.scalar.activation(dst, ps[:], IDENT, bias=bias_col)
                # Stores: quarters in phase 1 (early store stream) and in the
                # final bq-major stretch (keeps the ring drained for the
                # tail); halves in the ht-major middle. All on the sync
                # queue: scalar-queue stores delay ACT drain dispatches
                # (measured as end-of-stream PE stalls).
                if unit < 6 * BQ or unit >= n_units - 12:
                    nc.sync.dma_start(out_ap[ht][:, b0:b0 + 1024], out_sb[:, b0:b0 + 1024])
                elif bq == 1:
                    nc.sync.dma_start(out_ap[ht][:, 0:2048], out_sb[:, 0:2048])
                elif bq == 3:
                    nc.sync.dma_start(out_ap[ht][:, 2048:BS], out_sb[:, 2048:BS])

    _dedup_ldweights(nc)
    nc.compile()
    return nc


def kernel(x, W_gate, b_gate, W_exp, b_exp):
    global _LAST_RESULTS
    from concourse.bass_utils import run_bass_kernel_spmd

    config = CONFIG
    in_np = np.float16 if config == "f16" else np.float32

    x = np.asarray(x, dtype=np.float32)
    W_exp = np.asarray(W_exp, dtype=np.float32)
    b_exp = np.asarray(b_exp, dtype=np.float32)

    w_sum = W_exp.sum(axis=1).reshape(N, D)                    # [2048, 256]
    # device layout [P(i), KO, N]: wt[p, ko, n] = W_sum.T[ko*128+p, n]
    wt = np.ascontiguousarray(
        w_sum.T.reshape(KO, P, N).transpose(1, 0, 2).astype(in_np))
    b_sum = b_exp.sum(axis=1).reshape(N)                       # [2048]
    biasp = np.ascontiguousarray(b_sum.reshape(HT, P).T)       # [128, 16]
    xt = np.ascontiguousarray(x.T.astype(in_np))               # [256, 32768]

    in_maps = [
        {
            "xt": np.ascontiguousarray(xt[:, c * BS:(c + 1) * BS]),
            "wt": wt,
            "biasp": biasp,
        }
        for c in range(NCORES)
    ]

    if config not in _NC_CACHE:
        _NC_CACHE[config] = _build_nc(config)
    res = run_bass_kernel_spmd(_NC_CACHE[config], in_maps, core_ids=list(range(NCORES)))
    _LAST_RESULTS = res
    out_t = np.concatenate([r["out"] for r in res.results], axis=1)  # [2048, 32768]
    return np.ascontiguousarray(out_t.T.astype(np.float32))

